# revision 4
# baseline (speedup 1.0000x reference)
"""2D DCT-II (ortho) over the last two axes of x[8, 32, 512, 512] (f32),
data-parallel across 8 NeuronCores (one batch element per core).

Per core, per image: Y = D @ X @ D^T via the double even/odd (quad)
split: with A_a = D[a::2, :256] and X_ab the +/- fold of X over both
axes, Y[2k+a, 2l+b] = (A_a X_ab A_b^T)[k, l]. Every contraction is
256 long — half the tensor-engine work of the single-split version.

Row fold needs rows i and 511-i on the same partition: the low half
loads rows 2p+s on partition p (4KB contiguous HBM segments, HWDGE);
the high half is fetched with two gpsimd indirect row gathers
(idx = 511-2p-s, per-image base baked into the index input). The
column fold is a reversed free-dim DVE op as usual.

Stage-2 matmuls write PSUM with a stride-2 free AP (even/odd column
quadrants interleave in PSUM), so the PSUM->SBUF output copy is
contiguous, and the row-parity interleave is free via the output
DMA's access pattern (row u = 256*kb + 2p + a).

All matmul operands are bf16 (f32r weight loads are 4B/col and made
earlier versions ldweights-bound; bf16 loads hide under the matmuls).
"""
import numpy as np
import ml_dtypes

import concourse.bass as bass
import concourse.mybir as mybir
import concourse.tile as tile
from concourse.bass import IndirectOffsetOnAxis
from concourse.bass_utils import run_bass_kernel_spmd

P = 128
N = 512
H = N // 2          # 256
NIMG = 32
NCORES = 8

_MAX_WAITS = 1


def _split_excess_waits(nc):
    """walrus CoreV3 codegen rejects instructions carrying several sem
    waits; hoist excess waits onto preceding same-engine NoOps."""
    for f in nc.m.functions:
        for bb in f.blocks:
            insts = bb.instructions
            i = 0
            while i < len(insts):
                inst = insts[i]
                si = inst.sync_info
                if si is not None and si.on_wait and len(si.on_wait) > _MAX_WAITS:
                    waits = list(si.on_wait)
                    keep = waits[-_MAX_WAITS:]
                    hoist = waits[:-_MAX_WAITS]
                    nops = []
                    for w in hoist:
                        nop = mybir.InstNoOp(
                            name=nc.get_next_instruction_name(), ins=[], outs=[])
                        nop.engine = inst.engine
                        nop.sync_info = mybir.SyncInfo(on_wait=[w], on_update=[])
                        nops.append(nop)
                    si.on_wait = keep
                    for off, nop in enumerate(nops):
                        insts.insert(i + off, nop)
                    i += len(nops)
                i += 1


def _consts():
    k = np.arange(N)[:, None]
    j = np.arange(N)[None, :]
    D = np.cos(np.pi * (2 * j + 1) * k / (2.0 * N))
    D *= np.sqrt(2.0 / N)
    D[0] *= 1.0 / np.sqrt(2.0)
    A = [D[0::2, :H], D[1::2, :H]]          # [256 k, 256 i]
    # stage-1 rhs: A_a^T rows in i = 2p+s order -> [128, 2, 256]
    r1 = [np.ascontiguousarray(
        A[a].T.reshape(P, 2, H).astype(ml_dtypes.bfloat16)) for a in range(2)]
    # stage-2 rhs: A_b^T rows in j = jb*128+p order -> [128, 2, 256]
    r2 = [np.ascontiguousarray(
        A[b].T.reshape(2, P, H).transpose(1, 0, 2).astype(ml_dtypes.bfloat16))
        for b in range(2)]
    # gather indices: idx[p, img, s] = img*512 + 511 - 2p - s
    p = np.arange(P)[:, None, None]
    m = np.arange(NIMG)[None, :, None]
    s = np.arange(2)[None, None, :]
    idx = (m * N + N - 1 - 2 * p - s).astype(np.uint32)
    return r1[0], r1[1], r2[0], r2[1], np.ascontiguousarray(idx)


def _build():
    nc = bass.Bass()
    f32 = mybir.dt.float32
    bf16 = mybir.dt.bfloat16
    u32 = mybir.dt.uint32
    x_d = nc.dram_tensor("x", [NIMG, N, N], f32, kind="ExternalInput")
    r1a_d = nc.dram_tensor("r1a", [P, 2, H], bf16, kind="ExternalInput")
    r1b_d = nc.dram_tensor("r1b", [P, 2, H], bf16, kind="ExternalInput")
    r2a_d = nc.dram_tensor("r2a", [P, 2, H], bf16, kind="ExternalInput")
    r2b_d = nc.dram_tensor("r2b", [P, 2, H], bf16, kind="ExternalInput")
    idx_d = nc.dram_tensor("idx", [P, NIMG, 2], u32, kind="ExternalInput")
    y_d = nc.dram_tensor("y", [NIMG, N, N], f32, kind="ExternalOutput")

    x_flat = x_d.rearrange("m r c -> (m r) c")

    with tile.TileContext(nc) as tc:
        with (
            tc.tile_pool(name="const", bufs=1) as cpool,
            tc.tile_pool(name="xp", bufs=6) as xp,
            tc.tile_pool(name="cf", bufs=3) as cfp,
            tc.tile_pool(name="qf", bufs=3) as qfp,
            tc.tile_pool(name="zp", bufs=3) as zp,
            tc.tile_pool(name="yp", bufs=4) as yp,
            tc.tile_pool(name="ps", bufs=4, space="PSUM") as ps1p,
            tc.tile_pool(name="ps2", bufs=4, space="PSUM") as ps2p,
        ):
            r1a_t = cpool.tile([P, 2, H], bf16, tag="r1a")
            r1b_t = cpool.tile([P, 2, H], bf16, tag="r1b")
            r2a_t = cpool.tile([P, 2, H], bf16, tag="r2a")
            r2b_t = cpool.tile([P, 2, H], bf16, tag="r2b")
            r1 = [r1a_t, r1b_t]
            r2 = [r2a_t, r2b_t]
            for t, d in ((r1[0], r1a_d), (r1[1], r1b_d),
                         (r2[0], r2a_d), (r2[1], r2b_d)):
                nc.sync.dma_start(t[:], d[:])
            idx_t = cpool.tile([P, NIMG, 2], u32, tag="idx")
            nc.sync.dma_start(idx_t[:], idx_d[:])

            for img in range(NIMG):
                # [:, 0:2] rows 2p+s; [:, 2:4] rows 511-2p-s
                x_sb = xp.tile([P, 4, N], f32)
                nc.sync.dma_start(
                    x_sb[:, 0:2, :].rearrange("p s c -> p (s c)"),
                    x_d[img, 0:H].rearrange("(p s) c -> p (s c)", p=P))
                for s in range(2):
                    nc.gpsimd.indirect_dma_start(
                        x_sb[:, 2 + s, :], None, x_flat,
                        IndirectOffsetOnAxis(ap=idx_t[:, img, s:s + 1], axis=0))

                # column fold (f32 -> bf16): cle, clo, che, cho [p, s, j]
                cf = cfp.tile([P, 4, 2, H], bf16)
                lo, hi = x_sb[:, 0:2, :], x_sb[:, 2:4, :]
                lor = x_sb[:, 0:2, N - 1:H - 1:-1]
                hir = x_sb[:, 2:4, N - 1:H - 1:-1]
                nc.vector.tensor_add(cf[:, 0], lo[:, :, 0:H], lor)
                nc.vector.tensor_sub(cf[:, 1], lo[:, :, 0:H], lor)
                nc.vector.tensor_add(cf[:, 2], hi[:, :, 0:H], hir)
                nc.vector.tensor_sub(cf[:, 3], hi[:, :, 0:H], hir)

                # row fold (bf16, contiguous): quad[a, b][p, s, j]
                quad = qfp.tile([P, 2, 2, 2, H], bf16)
                nc.vector.tensor_add(quad[:, 0, 0], cf[:, 0], cf[:, 2])
                nc.vector.tensor_sub(quad[:, 1, 0], cf[:, 0], cf[:, 2])
                nc.vector.tensor_add(quad[:, 0, 1], cf[:, 1], cf[:, 3])
                nc.vector.tensor_sub(quad[:, 1, 1], cf[:, 1], cf[:, 3])

                # stage 1: Z_ab = (A_a X_ab)^T; z[a, b][p, jb, k]
                z_sb = zp.tile([P, 2, 2, 2, H], bf16)
                for a in range(2):
                    for b in range(2):
                        pz = ps1p.tile([P, 2, H], f32, tag="ps1")
                        for jb in range(2):
                            for s in range(2):
                                nc.tensor.matmul(
                                    pz[:, jb, :],
                                    quad[:, a, b, s, jb * P:(jb + 1) * P],
                                    r1[a][:, s, :],
                                    start=(s == 0),
                                    stop=(s == 1),
                                )
                        nc.scalar.copy(z_sb[:, a, b], pz[:])

                # stage 2: Y quadrant rows; psum interleaved over b via
                # stride-2 writes; py partition p holds row 256*kb+2p+a.
                y_sb = yp.tile([P, 2, 2, N], f32)
                for a in range(2):
                    for kb in range(2):
                        py = ps2p.tile([P, N], f32, tag="ps2")
                        for b in range(2):
                            for jb in range(2):
                                nc.tensor.matmul(
                                    py[:, b:N:2],
                                    z_sb[:, a, b, jb, kb * P:(kb + 1) * P],
                                    r2[b][:, jb, :],
                                    start=(jb == 0),
                                    stop=(jb == 1),
                                )
                        if a == 0 and kb == 0:
                            nc.vector.tensor_copy(y_sb[:, kb, a, :], py[:])
                        else:
                            nc.scalar.copy(y_sb[:, kb, a, :], py[:])
                nc.sync.dma_start(
                    y_d[img].rearrange("(kb p u2) v -> p kb (u2 v)", p=P, u2=2),
                    y_sb[:].rearrange("p kb a v -> p kb (a v)"))

    _split_excess_waits(nc)
    return nc


_CACHE = {}


def _get_nc():
    if "nc" not in _CACHE:
        _CACHE["nc"] = _build()
    return _CACHE["nc"]


def _in_maps(x):
    r1a, r1b, r2a, r2b, idx = _consts()
    return [{"x": x[i], "r1a": r1a, "r1b": r1b, "r2a": r2a, "r2b": r2b,
             "idx": idx} for i in range(NCORES)]


def kernel(x):
    x = np.ascontiguousarray(np.asarray(x, dtype=np.float32))
    assert x.shape == (NCORES, NIMG, N, N), x.shape
    nc = _get_nc()
    res = run_bass_kernel_spmd(nc, _in_maps(x), core_ids=list(range(NCORES)))
    out = np.stack([res.results[i]["y"] for i in range(NCORES)], axis=0)
    return out.astype(np.float32)
